# revision 1
# baseline (speedup 1.0000x reference)
"""DigitalMapper kernel for 8 trn2 NeuronCores.

Math: reference computes  out = (x @ softmax(W, axis=1).T) > 0.5  with
x in {0,1}.  Let E = exp(W) (row-unnormalized).  Then

  out[b,o] > 0.5
    <=>  sum_i x[b,i]*E[o,i] / sum_i E[o,i] > 0.5
    <=>  sum_i (x[b,i] - 0.5) * E[o,i] > 0

so the softmax divide, the row-max subtraction and the per-column
threshold all fold into a single zero-threshold on a centered matmul.
(The row-max factor exp(m_o) scales a whole column positively - sign
is unchanged; |W| <= ~5.5 so exp never overflows fp32.)

Sharding: 4 batch-groups x 2 out-feature-groups across 8 cores.  Each
core gets x.T[:, bg*1024:...] and W.T[:, og*1024:...] (host does only
transpose/slice; subtract/exp/matmul/threshold all run on device) and
produces a [1024, 1024] block of the output.
"""

import sys

sys.path.insert(0, "/opt/trn_rl_repo")

import numpy as np

BATCH, IN_F, OUT_F = 4096, 2048, 2048
N_CORES = 8
BG, OG = 4, 2  # batch groups x out-feature groups
B_PER = BATCH // BG  # 1024 batch rows per core
O_PER = OUT_F // OG  # 1024 out features per core
P = 128
KT = IN_F // P  # 16 contraction tiles
MT = B_PER // P  # 8 output row tiles per core
NFREE = 512  # matmul moving free dim (one PSUM bank of fp32)
NO = O_PER // NFREE  # 2 n-chunks

_COMPILED = {}


def _patch_tile_drain():
    """walrus in this container allows only ONE sem-wait per CTRL (Drain/NOP)
    instruction; Tile's kernel-tail drain aggregates one wait per live
    semaphore.  Split the waits across a chain of SP nops."""
    import concourse.mybir as mybir
    import concourse.tile as tile_mod
    from concourse.vector_clock import ScopedClock

    if getattr(tile_mod.TileContext, "_drain_split_patched", False):
        return

    def _drain_and_barrier_split(self, tick_clock, wait_clock):
        nc = self.nc
        drain_inst = nc.sync.drain()
        wait_clock.add_sem_waits(
            drain_inst.ins, ScopedClock({None: tick_clock.global_clock})
        )
        si = drain_inst.ins.sync_info
        waits = list(si.on_wait) if si is not None else []
        if len(waits) > 1:
            si.on_wait.clear()
            si.on_wait.extend(waits[:1])
            for w in waits[1:]:
                nop = nc.sync.nop(nofuse=True)
                if nop.ins.sync_info is None:
                    nop.ins.sync_info = mybir.SyncInfo(on_wait=[], on_update=[])
                nop.ins.sync_info.on_wait.append(w)
        nc.all_engine_barrier()
        assert self.sems is not None
        popped = nc._tile_sem_poison_stack.pop()
        assert popped is self._sem_poison
        nc.clear_and_free_semaphores(list(self.sems.allocated().values()))
        nc.all_engine_barrier()

    tile_mod.TileContext._drain_and_barrier = _drain_and_barrier_split
    tile_mod.TileContext._drain_split_patched = True


def _split_multi_waits(nc):
    """walrus here allows very few sem-waits per instruction.  Hoist extra
    waits onto same-engine NOPs placed immediately before the instruction
    (same blocking point, engine executes in order).  DMA-queue instructions
    keep their waits - their sync runs through the DGE queues."""
    import concourse.mybir as mybir

    n = 0
    for f in nc.m.functions:
        for bb in f.blocks:
            new_insts = []
            for inst in bb.instructions:
                si = inst.sync_info
                if si is not None and si.on_wait and len(si.on_wait) > 1:
                    waits = list(si.on_wait)
                    si.on_wait.clear()
                    si.on_wait.append(waits[0])
                    for w in waits[1:]:
                        n += 1
                        new_insts.append(
                            mybir.InstNoOp(
                                name=f"wsplit-{n}",
                                opcode="NoOp",
                                engine=inst.engine,
                                sync_info=mybir.SyncInfo(on_wait=[w], on_update=[]),
                                bass_nofuse=True,
                            )
                        )
                new_insts.append(inst)
            if n:
                try:
                    bb.instructions[:] = new_insts
                except TypeError:
                    bb.instructions = new_insts
    return n


def _build(mm_dtype_name: str = "float32r", split_waits: bool = True,
           repeats: int = 1, correction=False, grouped: bool = False):
    """One core's SPMD program.

    correction=False:  single fp32r matmul pass (PE ~56us/core).
    correction="bf16": fp32r pass on Ehi=round_f32r(exp(W)) plus a bf16
        pass on dE=exp(W)-Ehi (xb=+-1 is exact in both dtypes), which
        restores ~full-fp32 matmul accuracy at 2 cyc/row (PE ~110us).
    correction="f32r": same two-pass scheme but dE is kept in fp32r:
        simpler (no bf16 copies of xb), and the residual is even more
        precise (measured 3 bit-flips vs exact fp64 on 8.4M outputs).
    """
    if correction is True:
        correction = "bf16"
    # "mixed"  = single fp32r pass but with bf16 stationary (exact +-1)
    # "mixed8" = fp8dr correction with bf16 hi-pass stationary
    hi_xb_bf16 = False  # mixed 32/16-bit matmul inputs: rejected by walrus
    nsplit = correction == "fp8dr2"
    if nsplit:
        correction = "fp8dr"
    import concourse.bass as bass
    import concourse.mybir as mybir
    import concourse.tile as tile

    _patch_tile_drain()

    f32 = mybir.dt.float32
    bf16 = mybir.dt.bfloat16
    u8 = mybir.dt.uint8
    mm_dt = getattr(mybir.dt, mm_dtype_name)
    Alu = mybir.AluOpType
    Act = mybir.ActivationFunctionType
    B2 = B_PER // 2  # batch columns per half

    nc = bass.Bass()
    xt = nc.dram_tensor("xt", [IN_F, B_PER], u8, kind="ExternalInput")
    wt = nc.dram_tensor("wt", [IN_F, O_PER], f32, kind="ExternalInput")
    # 0/1 output is exact in uint8 - quarters the store DMA; host upcasts
    out = nc.dram_tensor("out", [B_PER, O_PER], u8, kind="ExternalOutput")

    with tile.TileContext(nc) as tc:
        with (
            tc.tile_pool(name="xu", bufs=1) as xu_pool,
            tc.tile_pool(name="wr", bufs=4) as wr_pool,
            tc.tile_pool(name="eh", bufs=1) as eh_pool,
            tc.tile_pool(name="dl", bufs=1) as dl_pool,
            tc.tile_pool(name="xb", bufs=1) as xb_pool,
            tc.tile_pool(name="d8", bufs=1) as d8_pool,
            tc.tile_pool(name="x8", bufs=1) as x8_pool,
            tc.tile_pool(name="ps", bufs=1, space="PSUM") as ps_pool,
            tc.tile_pool(name="ot", bufs=3) as ot_pool,
        ):
          for _rep in range(repeats):
            if _rep == 0:
                # touch Exp immediately so the ~2.7us ACT table load
                # overlaps the first input DMAs instead of the first matmul
                warm = wr_pool.tile([P, 1], f32, name="warm", tag="warm")
                nc.vector.memset(warm[:], 0.0)
                nc.scalar.activation(warm[:], warm[:], Act.Exp)
            xu, ehi, dlo = [], [], []
            for k in range(KT):
                xk = xu_pool.tile([P, B_PER], u8, name=f"xu{k}", tag=f"xu{k}")
                nc.sync.dma_start(xk[:], xt[k * P : (k + 1) * P, :])
                xu.append(xk)
                wr = wr_pool.tile([P, O_PER], f32, name="wr", tag="wr")
                ek = eh_pool.tile([P, O_PER], mm_dt, name=f"e{k}", tag=f"e{k}")
                if correction == "fp8dr":
                    # k-pairs share a [P, 2, O] fp8 tile; dE scaled by 2^8
                    # (power of two - products with xb*2^-8 stay exact)
                    if k % 2 == 0:
                        dk = d8_pool.tile(
                            [P, 2, O_PER], mybir.dt.float8e4,
                            name=f"d8_{k // 2}", tag=f"d8_{k // 2}",
                        )
                        dlo.append(dk)
                    else:
                        dk = dlo[-1]
                else:
                    d_dt = bf16 if correction == "bf16" else mm_dt
                    dk = (
                        dl_pool.tile([P, O_PER], d_dt, name=f"d{k}", tag=f"d{k}")
                        if correction
                        else None
                    )
                # geometric ramp on the first weight tile (128/128/256/512
                # cols) so exp and the first matmuls start ~1us after kernel
                # launch instead of waiting for the full 512KB row-block
                bounds = [0, 128, 256, 512, 1024] if k == 0 else [0, O_PER]
                for q in range(len(bounds) - 1):
                    sl = slice(bounds[q], bounds[q + 1])
                    nc.sync.dma_start(wr[:, sl], wt[k * P : (k + 1) * P, sl])
                    if correction == "fp8dr":
                        # engine balance: ACT does exp+round (1.7us/k), the
                        # idle GpSimd takes the f32 subtract, DVE does the
                        # scaled fp8 cast - keeps every engine under the PE
                        # pace in the DMA-fed first half
                        nc.scalar.activation(wr[:, sl], wr[:, sl], Act.Exp)
                        nc.scalar.copy(ek[:, sl], wr[:, sl])  # rounds -> f32r
                        nc.gpsimd.tensor_tensor(
                            wr[:, sl], wr[:, sl], ek[:, sl], Alu.subtract
                        )
                        nc.vector.tensor_scalar(
                            dk[:, k % 2, sl], wr[:, sl], 256.0, None, Alu.mult
                        )
                    elif correction:
                        nc.scalar.activation(wr[:, sl], wr[:, sl], Act.Exp)
                        nc.scalar.copy(ek[:, sl], wr[:, sl])  # rounds -> f32r
                        nc.vector.tensor_tensor(
                            dk[:, sl], wr[:, sl], ek[:, sl], Alu.subtract
                        )
                    else:
                        nc.scalar.activation(ek[:, sl], wr[:, sl], Act.Exp)
                if correction and correction != "fp8dr":
                    dlo.append(dk)
                ehi.append(ek)

            if nsplit:
                # 8-wide m over one 512-col n-slice at a time: pass A's PE
                # pace (8 hi + 4 pair MMs per k ~ 2.1us) matches the DMA+
                # exp+round+sub+cast prep cadence, pass B runs PE-dense.
                xbr8, x88 = [], []
                for k in range(KT):
                    xb_k = xb_pool.tile(
                        [P, B_PER], mm_dt, name=f"xf{k}", tag=f"xf{k}"
                    )
                    nc.vector.tensor_scalar(
                        xb_k[:], xu[k][:], 2.0, 1.0, Alu.mult, Alu.subtract
                    )
                    xbr8.append(xb_k)
                    if k % 2 == 0:
                        x8_k = x8_pool.tile(
                            [P, 2, B_PER], mybir.dt.float8e4,
                            name=f"xp{k // 2}", tag=f"xp{k // 2}",
                        )
                        x88.append(x8_k)
                    nc.vector.tensor_scalar(
                        x88[-1][:, k % 2, :], xu[k][:],
                        2.0 ** -7, 2.0 ** -8, Alu.mult, Alu.subtract,
                    )

                def hi8(k, m, n, ps):
                    nc.tensor.matmul(
                        ps[:],
                        xbr8[k][:, m * P : (m + 1) * P],
                        ehi[k][:, n * NFREE : (n + 1) * NFREE],
                        start=(k == 0),
                        stop=False,
                    )

                def lo8(k, m, n, ps):
                    t = (k - 1) // 2
                    nc.tensor.matmul(
                        ps[:],
                        x88[t][:, :, m * P : (m + 1) * P],
                        dlo[t][:, :, n * NFREE : (n + 1) * NFREE],
                        start=False,
                        stop=(t == KT // 2 - 1),
                        perf_mode=mybir.MatmulPerfMode.DoubleRow,
                    )

                def evict8(m, n, ps):
                    otm = ot_pool.tile([P, NFREE], f32, name="otm", tag="otm")
                    nc.vector.tensor_scalar(
                        otm[:], ps[:], 0.0, None, Alu.is_gt
                    )
                    nc.sync.dma_start(
                        out[m * P : (m + 1) * P, n * NFREE : (n + 1) * NFREE],
                        otm[:],
                    )

                for n in range(NO):
                    pss = {
                        m: ps_pool.tile(
                            [P, NFREE], f32, name=f"pn_{m}", tag=f"pn_{m}"
                        )
                        for m in range(MT)
                    }
                    if n == 0:
                        for k in range(KT):
                            for m in range(MT):
                                hi8(k, m, n, pss[m])
                            if k % 2 == 1:
                                for m in range(MT):
                                    lo8(k, m, n, pss[m])
                        for m in range(MT):
                            evict8(m, n, pss[m])
                    else:
                        for m in range(MT):
                            for k in range(KT):
                                hi8(k, m, n, pss[m])
                                if k % 2 == 1:
                                    lo8(k, m, n, pss[m])
                            evict8(m, n, pss[m])
                continue_reps = True
            if not nsplit:
              for half in range(2):
                ms = range(half * 4, half * 4 + 4)
                xbr, xbb = [], []
                xb_dt = bf16 if hi_xb_bf16 else mm_dt
                for k in range(KT):
                    xb_k = xb_pool.tile([P, B2], xb_dt, name=f"xb{k}", tag=f"xb{k}")
                    # x in {0,1} -> xb = 2x-1 in {-1,+1}, exact in any fp dtype
                    nc.vector.tensor_scalar(
                        xb_k[:], xu[k][:, half * B2 : (half + 1) * B2],
                        2.0, 1.0, Alu.mult, Alu.subtract,
                    )
                    xbr.append(xb_k)
                    if correction == "fp8dr":
                        if k % 2 == 0:
                            x8_k = x8_pool.tile(
                                [P, 2, B2], mybir.dt.float8e4,
                                name=f"x8_{k // 2}", tag=f"x8_{k // 2}",
                            )
                            xbb.append(x8_k)
                        # xb8 = (2x-1)*2^-8, exact in fp8e4 (denormal 2^-8)
                        nc.vector.tensor_scalar(
                            xbb[-1][:, k % 2, :],
                            xu[k][:, half * B2 : (half + 1) * B2],
                            2.0 ** -7, 2.0 ** -8, Alu.mult, Alu.subtract,
                        )
                    if correction == "bf16":
                        xbb_k = xb_pool.tile(
                            [P, B2], bf16, name=f"xc{k}", tag=f"xc{k}"
                        )
                        nc.scalar.copy(xbb_k[:], xb_k[:])
                        xbb.append(xbb_k)

                pss = {}
                for m in ms:
                    pss[m] = ps_pool.tile(
                        [P, O_PER], f32, name=f"ps_{m % 4}", tag=f"ps_{m % 4}"
                    )

                def hi_mms(k, m):
                    lhsT = xbr[k][:, (m % 4) * P : (m % 4 + 1) * P]
                    for n in range(NO):
                        nc.tensor.matmul(
                            pss[m][:, n * NFREE : (n + 1) * NFREE],
                            lhsT,
                            ehi[k][:, n * NFREE : (n + 1) * NFREE],
                            start=(k == 0),
                            stop=(k == KT - 1 and not correction),
                        )

                def lo_mms(k, m):
                    if correction == "fp8dr":
                        if k % 2 == 0:
                            return  # one DoubleRow MM per completed k-pair
                        t = (k - 1) // 2
                        lhsTb = xbb[t][:, :, (m % 4) * P : (m % 4 + 1) * P]
                        for n in range(NO):
                            nc.tensor.matmul(
                                pss[m][:, n * NFREE : (n + 1) * NFREE],
                                lhsTb,
                                dlo[t][:, :, n * NFREE : (n + 1) * NFREE],
                                start=False,
                                stop=(t == KT // 2 - 1),
                                perf_mode=mybir.MatmulPerfMode.DoubleRow,
                            )
                        return
                    src_xb = xbb if correction == "bf16" else xbr
                    lhsTb = src_xb[k][:, (m % 4) * P : (m % 4 + 1) * P]
                    for n in range(NO):
                        nc.tensor.matmul(
                            pss[m][:, n * NFREE : (n + 1) * NFREE],
                            lhsTb,
                            dlo[k][:, n * NFREE : (n + 1) * NFREE],
                            start=False,
                            stop=(k == KT - 1),
                        )

                def emit_mms(k, m):
                    hi_mms(k, m)
                    if correction:
                        lo_mms(k, m)

                def evict(m, pipelined=False):
                    otm = ot_pool.tile([P, O_PER], u8, name="otm", tag="otm")
                    row = half * 4 * P + (m % 4) * P
                    if pipelined:
                        # per-n-slice evict+store so the final DMA only
                        # trails the last psum bank, not the whole row
                        for n in range(NO):
                            sl = slice(n * NFREE, (n + 1) * NFREE)
                            nc.vector.tensor_scalar(
                                otm[:, sl], pss[m][:, sl], 0.0, None, Alu.is_gt
                            )
                            nc.sync.dma_start(out[row : row + P, sl], otm[:, sl])
                    else:
                        nc.vector.tensor_scalar(
                            otm[:], pss[m][:], 0.0, None, Alu.is_gt
                        )
                        nc.sync.dma_start(out[row : row + P, :], otm[:])

                if half == 0:
                    # k-outer: consume E[k] in DMA/exp arrival order
                    for k in range(KT):
                        if grouped and correction:
                            # same-dtype runs: a bf16 MM directly after an
                            # fp32(r) MM can't use fast-weight-load (FWL
                            # guard on LastMatmultFP32HI) - group passes
                            for m in ms:
                                hi_mms(k, m)
                            for m in ms:
                                lo_mms(k, m)
                        else:
                            for m in ms:
                                emit_mms(k, m)
                    for m in ms:
                        evict(m)
                else:
                    # all tiles resident now: m-outer so each m's psum
                    # finishes early and eviction/out-DMA pipelines
                    for m in ms:
                        if grouped and correction:
                            for k in range(KT):
                                hi_mms(k, m)
                            for k in range(KT):
                                lo_mms(k, m)
                        else:
                            for k in range(KT):
                                emit_mms(k, m)
                        evict(m, pipelined=True)

    if split_waits:
        _split_multi_waits(nc)
    return nc


def _get_compiled(mm_dtype_name: str = "float32r", correction=False):
    key = (mm_dtype_name, correction)
    if key not in _COMPILED:
        _COMPILED[key] = _build(mm_dtype_name, correction=correction)
    return _COMPILED[key]


def kernel(x: np.ndarray, raw_weight: np.ndarray, _mm_dtype: str = "float32r",
           _correction="fp8dr", _trace: bool = False):
    from concourse.bass_utils import run_bass_kernel_spmd

    nc = _get_compiled(_mm_dtype, _correction)

    # materialize as numpy first (inputs may arrive as jax arrays)
    x = np.asarray(x)
    raw_weight = np.asarray(raw_weight)

    # x is exactly 0.0/1.0; uint8 encodes it losslessly and quarters the DMA
    xT = np.ascontiguousarray(x.T.astype(np.uint8))
    wT = np.ascontiguousarray(raw_weight.T).astype(np.float32, copy=False)

    in_maps = []
    for c in range(N_CORES):
        bg, og = divmod(c, OG)
        in_maps.append(
            {
                "xt": np.ascontiguousarray(xT[:, bg * B_PER : (bg + 1) * B_PER]),
                "wt": np.ascontiguousarray(wT[:, og * O_PER : (og + 1) * O_PER]),
            }
        )

    res = run_bass_kernel_spmd(
        nc, in_maps, core_ids=list(range(N_CORES)), trace=_trace
    )

    full = np.empty((BATCH, OUT_F), dtype=x.dtype)
    for c in range(N_CORES):
        bg, og = divmod(c, OG)
        full[bg * B_PER : (bg + 1) * B_PER, og * O_PER : (og + 1) * O_PER] = (
            res.results[c]["out"]
        )
    if _trace:
        kernel.last_results = res
    return full



# revision 4
# speedup vs baseline: 1.0779x; 1.0779x over previous
"""DigitalMapper kernel for 8 trn2 NeuronCores.

Math: reference computes  out = (x @ softmax(W, axis=1).T) > 0.5  with
x in {0,1}.  Let E = exp(W) (row-unnormalized).  Then

  out[b,o] > 0.5
    <=>  sum_i x[b,i]*E[o,i] / sum_i E[o,i] > 0.5
    <=>  sum_i (x[b,i] - 0.5) * E[o,i] > 0

so the softmax divide, the row-max subtraction and the per-column
threshold all fold into a single zero-threshold on a centered matmul.
(The row-max factor exp(m_o) scales a whole column positively - sign
is unchanged; |W| <= ~5.5 so exp never overflows fp32.)

Sharding: 4 batch-groups x 2 out-feature-groups across 8 cores.  Each
core gets x.T[:, bg*1024:...] and W.T[:, og*1024:...] (host does only
transpose/slice; subtract/exp/matmul/threshold all run on device) and
produces a [1024, 1024] block of the output.

Numerics: a single fp32r pass measures 256/8.4M sign flips vs the fp32
reference (rel err 7.8e-3, comfortably under the 2e-2 gate), so no
correction pass is run.  PE work is exactly 131072 rows = 54.6us/core.

Schedule (single-pass):
  phase A: out-cols 0:512 for ALL 8 m-tiles, k-outer, using all 8 PSUM
    banks.  Per k-tile it only needs x (128KB) + half of W (256KB), so
    DMA feeds it at ~1.1us/k vs 1.7us/k of PE work -> PE-bound (the old
    4m x 2n half-split needed the full W per k and was DMA-bound).
  The n1 half of W streams in behind phase A's data and is exp'd as it
    lands; x and W-n0 use 2-k-pair DMAs to halve SP descriptor-gen
    serialization so the n1 stream starts early enough.
  phase A's evicts interleave into the k=15 matmuls (no A->B bubble).
  phase B: out-cols 512:1024, m-outer over resident tiles, pipelined
    per-m evicts; the last m runs as 2x256-wide chunks so the final
    evict+store tail after the last matmul is short.
"""

import sys

sys.path.insert(0, "/opt/trn_rl_repo")

import numpy as np

BATCH, IN_F, OUT_F = 4096, 2048, 2048
N_CORES = 8
BG, OG = 4, 2  # batch groups x out-feature groups
B_PER = BATCH // BG  # 1024 batch rows per core
O_PER = OUT_F // OG  # 1024 out features per core
P = 128
KT = IN_F // P  # 16 contraction tiles
MT = B_PER // P  # 8 output row tiles per core
NH = 512  # out-col half width (one PSUM bank of fp32)

_COMPILED = {}


def _patch_tile_drain():
    """walrus in this container allows only ONE sem-wait per CTRL (Drain/NOP)
    instruction; Tile's kernel-tail drain aggregates one wait per live
    semaphore.  Split the waits across a chain of SP nops."""
    import concourse.mybir as mybir
    import concourse.tile as tile_mod
    from concourse.vector_clock import ScopedClock

    if getattr(tile_mod.TileContext, "_drain_split_patched", False):
        return

    def _drain_and_barrier_split(self, tick_clock, wait_clock):
        nc = self.nc
        drain_inst = nc.sync.drain()
        wait_clock.add_sem_waits(
            drain_inst.ins, ScopedClock({None: tick_clock.global_clock})
        )
        si = drain_inst.ins.sync_info
        waits = list(si.on_wait) if si is not None else []
        if len(waits) > 1:
            si.on_wait.clear()
            si.on_wait.extend(waits[:1])
            for w in waits[1:]:
                nop = nc.sync.nop(nofuse=True)
                if nop.ins.sync_info is None:
                    nop.ins.sync_info = mybir.SyncInfo(on_wait=[], on_update=[])
                nop.ins.sync_info.on_wait.append(w)
        nc.all_engine_barrier()
        assert self.sems is not None
        popped = nc._tile_sem_poison_stack.pop()
        assert popped is self._sem_poison
        nc.clear_and_free_semaphores(list(self.sems.allocated().values()))
        nc.all_engine_barrier()

    tile_mod.TileContext._drain_and_barrier = _drain_and_barrier_split
    tile_mod.TileContext._drain_split_patched = True


def _split_multi_waits(nc):
    """walrus here allows very few sem-waits per instruction.  Hoist extra
    waits onto same-engine NOPs placed immediately before the instruction
    (same blocking point, engine executes in order).  DMA-queue instructions
    keep their waits - their sync runs through the DGE queues."""
    import concourse.mybir as mybir

    n = 0
    for f in nc.m.functions:
        for bb in f.blocks:
            new_insts = []
            for inst in bb.instructions:
                si = inst.sync_info
                if si is not None and si.on_wait and len(si.on_wait) > 1:
                    waits = list(si.on_wait)
                    si.on_wait.clear()
                    si.on_wait.append(waits[0])
                    for w in waits[1:]:
                        n += 1
                        new_insts.append(
                            mybir.InstNoOp(
                                name=f"wsplit-{n}",
                                opcode="NoOp",
                                engine=inst.engine,
                                sync_info=mybir.SyncInfo(on_wait=[w], on_update=[]),
                                bass_nofuse=True,
                            )
                        )
                new_insts.append(inst)
            if n:
                try:
                    bb.instructions[:] = new_insts
                except TypeError:
                    bb.instructions = new_insts
    return n


def _build(mm_dtype_name: str = "float32r", split_waits: bool = True,
           repeats: int = 1, correction=False, grouped: bool = False,
           tail_split: int = 2):
    """One core's SPMD program: single mm_dtype pass, no correction.

    (The correction machinery from the 80us baseline was dropped: the
    rel-err gate is 2e-2 ~ 1678 sign flips, a lone fp32r pass measures
    256 flips on hw.  correction/grouped/repeats args are accepted for
    test.py compat and must be falsy/1.)
    """
    assert not correction and repeats == 1 and not grouped
    import concourse.bass as bass
    import concourse.mybir as mybir
    import concourse.tile as tile

    _patch_tile_drain()

    f32 = mybir.dt.float32
    u8 = mybir.dt.uint8
    mm_dt = getattr(mybir.dt, mm_dtype_name)
    Alu = mybir.AluOpType
    Act = mybir.ActivationFunctionType

    nc = bass.Bass()
    xt = nc.dram_tensor("xt", [IN_F, B_PER], u8, kind="ExternalInput")
    wt = nc.dram_tensor("wt", [IN_F, O_PER], f32, kind="ExternalInput")
    # 0/1 output is exact in uint8 - quarters the store DMA; host upcasts
    out = nc.dram_tensor("out", [B_PER, O_PER], u8, kind="ExternalOutput")

    with tile.TileContext(nc) as tc:
        with (
            tc.tile_pool(name="xu", bufs=1) as xu_pool,
            tc.tile_pool(name="xb", bufs=1) as xb_pool,
            tc.tile_pool(name="wr", bufs=4) as wr_pool,
            tc.tile_pool(name="ek", bufs=1) as ek_pool,
            tc.tile_pool(name="ps", bufs=1, space="PSUM") as ps_pool,
            tc.tile_pool(name="ot", bufs=12) as ot_pool,
        ):
            # touch Exp immediately so the ACT table load overlaps the
            # first input DMAs instead of the first exp
            warm = wr_pool.tile([P, 1], f32, name="warm", tag="warm")
            nc.vector.memset(warm[:], 0.0)
            nc.scalar.activation(warm[:], warm[:], Act.Exp)

            ek = [
                ek_pool.tile([P, O_PER], mm_dt, name=f"e{k}", tag=f"e{k}")
                for k in range(KT)
            ]
            xb = [
                xb_pool.tile([P, B_PER], mm_dt, name=f"xb{k}", tag=f"xb{k}")
                for k in range(KT)
            ]

            # ---- input streams ----------------------------------------
            # SP order: [x k0, W-n0 k0 ramp, x k1, W-n0 k1, then x/W-n0
            # 2k-pairs] followed by the 16 per-k W-n1 DMAs.  Pairing keeps
            # SP's ~0.5-0.8us/DMA descriptor-gen off the critical path so
            # the n1 stream is fully issued before phase A's PE work ends.
            xup: list = []  # per-k APs into the x tiles

            def x_dma(k_pair):
                if k_pair == 0:
                    for k in (0, 1):
                        t = xu_pool.tile([P, B_PER], u8, name=f"xu{k}", tag=f"xu{k}")
                        nc.sync.dma_start(t[:], xt[k * P : (k + 1) * P, :])
                        xup.append(t[:])
                else:
                    t = xu_pool.tile(
                        [P, 2, B_PER], u8, name=f"xu{k_pair}", tag=f"xup{k_pair}"
                    )
                    r = k_pair * 2 * P
                    nc.sync.dma_start(t[:], xt[r : r + 2 * P, :])
                    xup.append(t[:, 0, :])
                    xup.append(t[:, 1, :])

            def w_n0_dma(k_pair):
                # k0: geometric ramp (128/128/256 cols) so the first exp and
                # matmuls start ~1us earlier than a full 256KB block allows
                if k_pair == 0:
                    for k, bounds in ((0, [0, 128, 256, NH]), (1, [0, NH])):
                        wrk = wr_pool.tile([P, NH], f32, name="wr", tag="wr")
                        for q in range(len(bounds) - 1):
                            sl = slice(bounds[q], bounds[q + 1])
                            nc.sync.dma_start(wrk[:, sl], wt[k * P : (k + 1) * P, sl])
                            nc.scalar.activation(ek[k][:, sl], wrk[:, sl], Act.Exp)
                else:
                    wrk = wr_pool.tile([P, 2, NH], f32, name="wrp", tag="wrp")
                    r = k_pair * 2 * P
                    nc.sync.dma_start(wrk[:], wt[r : r + 2 * P, 0:NH])
                    for j in (0, 1):
                        nc.scalar.activation(
                            ek[k_pair * 2 + j][:, 0:NH], wrk[:, j, :], Act.Exp
                        )

            for kp in range(KT // 2):
                x_dma(kp)
                w_n0_dma(kp)

            # n1 half of W: streams behind the phase-A data, exp'd on ACT
            # as it lands; consumed only by phase B.  The 2k-pair DMAs park
            # rows 2p/2p+1 on partition p - a permutation of the contraction
            # index that phase A's x-pairs share; the n1 stream must use the
            # SAME pairing so phase B's lhsT/rhs partitions line up too.
            for k in (0, 1):
                wrk = wr_pool.tile([P, NH], f32, name="wr1", tag="wr1")
                nc.sync.dma_start(wrk[:], wt[k * P : (k + 1) * P, NH:])
                nc.scalar.activation(ek[k][:, NH:], wrk[:], Act.Exp)
            for kp in range(1, KT // 2):
                wrk = wr_pool.tile([P, 2, NH], f32, name="wr1p", tag="wr1p")
                r = kp * 2 * P
                nc.sync.dma_start(wrk[:], wt[r : r + 2 * P, NH:])
                for j in (0, 1):
                    nc.scalar.activation(ek[kp * 2 + j][:, NH:], wrk[:, j, :], Act.Exp)

            # x -> 2x-1 in {-1,+1} (exact in any fp dtype).  k0 split in
            # halves so m0..3's lhsT is ready a few hundred ns earlier.
            for k in range(KT):
                if k == 0:
                    nc.vector.tensor_scalar(
                        xb[k][:, 0:NH], xup[k][:, 0:NH], 2.0, 1.0,
                        Alu.mult, Alu.subtract,
                    )
                    nc.vector.tensor_scalar(
                        xb[k][:, NH:], xup[k][:, NH:], 2.0, 1.0,
                        Alu.mult, Alu.subtract,
                    )
                else:
                    nc.vector.tensor_scalar(
                        xb[k][:], xup[k][:], 2.0, 1.0, Alu.mult, Alu.subtract
                    )

            # ---- compute ----------------------------------------------
            def evict(m, col0, ncols, psrc):
                otm = ot_pool.tile([P, ncols], u8, name="ot", tag=f"ot{ncols}")
                nc.vector.tensor_scalar(otm[:], psrc, 0.0, None, Alu.is_gt)
                nc.sync.dma_start(
                    out[m * P : (m + 1) * P, col0 : col0 + ncols], otm[:]
                )

            # phase A: n-cols 0:512, k-outer over all 8 m (8 PSUM banks).
            # At k=15 each m's evict is queued right after its last matmul
            # so DVE drains the banks while PE finishes the k-tile.
            psA = {
                m: ps_pool.tile([P, NH], f32, name=f"ps{m}", tag=f"ps{m}")
                for m in range(MT)
            }
            for k in range(KT):
                for m in range(MT):
                    nc.tensor.matmul(
                        psA[m][:],
                        xb[k][:, m * P : (m + 1) * P],
                        ek[k][:, 0:NH],
                        start=(k == 0),
                        stop=(k == KT - 1),
                    )
                    if k == KT - 1:
                        evict(m, 0, NH, psA[m][:])

            # phase B: n-cols 512:1024, m-outer, all tiles resident.
            for m in range(MT):
                psB = ps_pool.tile([P, NH], f32, name=f"ps{m}", tag=f"ps{m}")
                nw = NH // tail_split if m == MT - 1 else NH
                for c0 in range(0, NH, nw):
                    for k in range(KT):
                        nc.tensor.matmul(
                            psB[:, c0 : c0 + nw],
                            xb[k][:, m * P : (m + 1) * P],
                            ek[k][:, NH + c0 : NH + c0 + nw],
                            start=(k == 0),
                            stop=(k == KT - 1),
                        )
                    evict(m, NH + c0, nw, psB[:, c0 : c0 + nw])

    if split_waits:
        _split_multi_waits(nc)
    return nc


def _get_compiled(mm_dtype_name: str = "float32r", correction=False):
    key = (mm_dtype_name, correction)
    if key not in _COMPILED:
        _COMPILED[key] = _build(mm_dtype_name, correction=correction)
    return _COMPILED[key]


def kernel(x: np.ndarray, raw_weight: np.ndarray, _mm_dtype: str = "float32r",
           _correction=False, _trace: bool = False):
    from concourse.bass_utils import run_bass_kernel_spmd

    nc = _get_compiled(_mm_dtype, _correction)

    # materialize as numpy first (inputs may arrive as jax arrays)
    x = np.asarray(x)
    raw_weight = np.asarray(raw_weight)

    # x is exactly 0.0/1.0; uint8 encodes it losslessly and quarters the DMA
    xT = np.ascontiguousarray(x.T.astype(np.uint8))
    wT = np.ascontiguousarray(raw_weight.T).astype(np.float32, copy=False)

    in_maps = []
    for c in range(N_CORES):
        bg, og = divmod(c, OG)
        in_maps.append(
            {
                "xt": np.ascontiguousarray(xT[:, bg * B_PER : (bg + 1) * B_PER]),
                "wt": np.ascontiguousarray(wT[:, og * O_PER : (og + 1) * O_PER]),
            }
        )

    res = run_bass_kernel_spmd(
        nc, in_maps, core_ids=list(range(N_CORES)), trace=_trace
    )

    full = np.empty((BATCH, OUT_F), dtype=x.dtype)
    for c in range(N_CORES):
        bg, og = divmod(c, OG)
        full[bg * B_PER : (bg + 1) * B_PER, og * O_PER : (og + 1) * O_PER] = (
            res.results[c]["out"]
        )
    if _trace:
        kernel.last_results = res
    return full


# revision 14
# speedup vs baseline: 1.1223x; 1.0412x over previous
"""DigitalMapper kernel for 8 trn2 NeuronCores.

Math: reference computes  out = (x @ softmax(W, axis=1).T) > 0.5  with
x in {0,1}.  Let E = exp(W) (row-unnormalized).  Then

  out[b,o] > 0.5
    <=>  sum_i x[b,i]*E[o,i] / sum_i E[o,i] > 0.5
    <=>  sum_i (x[b,i] - 0.5) * E[o,i] > 0

so the softmax divide, the row-max subtraction and the per-column
threshold all fold into a single zero-threshold on a centered matmul.
(The row-max factor exp(m_o) scales a whole column positively - sign
is unchanged; |W| <= ~5.5 so exp never overflows fp32.)

Sharding: 4 batch-groups x 2 out-feature-groups across 8 cores.  Each
core gets x.T[:, bg*1024:...] and W.T[:, og*1024:...] (host does only
transpose/slice; subtract/exp/matmul/threshold all run on device) and
produces a [1024, 1024] block of the output.

Numerics: a single fp32r pass measures 256/8.4M sign flips vs the fp32
reference (rel err 7.8e-3, comfortably under the 2e-2 gate), so no
correction pass is run.  PE work is exactly 131072 rows = 54.6us/core.

Schedule (single-pass):
  phase A: out-cols 0:512 for ALL 8 m-tiles, k-outer, using all 8 PSUM
    banks.  Per k-tile it only needs x (128KB) + half of W (256KB), so
    DMA feeds it at ~1.1us/k vs 1.7us/k of PE work -> PE-bound (the old
    4m x 2n half-split needed the full W per k and was DMA-bound).
  The n1 half of W streams in behind phase A's data and is exp'd as it
    lands; x and W-n0 use 2-k-pair DMAs to halve SP descriptor-gen
    serialization so the n1 stream starts early enough.
  phase A's evicts interleave into the k=15 matmuls (no A->B bubble).
  phase B: out-cols 512:1024, m-outer over resident tiles, pipelined
    per-m evicts; the last m runs as 2x256-wide chunks so the final
    evict+store tail after the last matmul is short.
"""

import sys

sys.path.insert(0, "/opt/trn_rl_repo")

import numpy as np

BATCH, IN_F, OUT_F = 4096, 2048, 2048
N_CORES = 8
BG, OG = 4, 2  # batch groups x out-feature groups
B_PER = BATCH // BG  # 1024 batch rows per core
O_PER = OUT_F // OG  # 1024 out features per core
P = 128
KT = IN_F // P  # 16 contraction tiles
MT = B_PER // P  # 8 output row tiles per core
NH = 512  # out-col half width (one PSUM bank of fp32)

_COMPILED = {}


def _patch_tile_drain():
    """walrus in this container allows only ONE sem-wait per CTRL (Drain/NOP)
    instruction; Tile's kernel-tail drain aggregates one wait per live
    semaphore.  Split the waits across a chain of SP nops."""
    import concourse.mybir as mybir
    import concourse.tile as tile_mod
    from concourse.vector_clock import ScopedClock

    if getattr(tile_mod.TileContext, "_drain_split_patched", False):
        return

    def _drain_and_barrier_split(self, tick_clock, wait_clock):
        nc = self.nc
        drain_inst = nc.sync.drain()
        wait_clock.add_sem_waits(
            drain_inst.ins, ScopedClock({None: tick_clock.global_clock})
        )
        si = drain_inst.ins.sync_info
        waits = list(si.on_wait) if si is not None else []
        if len(waits) > 1:
            si.on_wait.clear()
            si.on_wait.extend(waits[:1])
            for w in waits[1:]:
                nop = nc.sync.nop(nofuse=True)
                if nop.ins.sync_info is None:
                    nop.ins.sync_info = mybir.SyncInfo(on_wait=[], on_update=[])
                nop.ins.sync_info.on_wait.append(w)
        nc.all_engine_barrier()
        assert self.sems is not None
        popped = nc._tile_sem_poison_stack.pop()
        assert popped is self._sem_poison
        nc.clear_and_free_semaphores(list(self.sems.allocated().values()))
        nc.all_engine_barrier()

    tile_mod.TileContext._drain_and_barrier = _drain_and_barrier_split
    tile_mod.TileContext._drain_split_patched = True


def _split_multi_waits(nc):
    """walrus here allows very few sem-waits per instruction.  Hoist extra
    waits onto same-engine NOPs placed immediately before the instruction
    (same blocking point, engine executes in order).  DMA-queue instructions
    keep their waits - their sync runs through the DGE queues."""
    import concourse.mybir as mybir

    n = 0
    for f in nc.m.functions:
        for bb in f.blocks:
            new_insts = []
            for inst in bb.instructions:
                si = inst.sync_info
                if si is not None and si.on_wait and len(si.on_wait) > 1:
                    waits = list(si.on_wait)
                    si.on_wait.clear()
                    si.on_wait.append(waits[0])
                    for w in waits[1:]:
                        n += 1
                        new_insts.append(
                            mybir.InstNoOp(
                                name=f"wsplit-{n}",
                                opcode="NoOp",
                                engine=inst.engine,
                                sync_info=mybir.SyncInfo(on_wait=[w], on_update=[]),
                                bass_nofuse=True,
                            )
                        )
                new_insts.append(inst)
            if n:
                try:
                    bb.instructions[:] = new_insts
                except TypeError:
                    bb.instructions = new_insts
    return n


def _build(mm_dtype_name: str = "float32r", split_waits: bool = True,
           repeats: int = 1, correction=False, grouped: bool = False,
           tail_split: int = 2):
    """One core's SPMD program: single mm_dtype pass, no correction.

    (The correction machinery from the 80us baseline was dropped: the
    rel-err gate is 2e-2 ~ 1678 sign flips, a lone fp32r pass measures
    256 flips on hw.  correction/grouped/repeats args are accepted for
    test.py compat and must be falsy/1.)
    """
    assert not correction and repeats == 1 and not grouped
    import concourse.bass as bass
    import concourse.mybir as mybir
    import concourse.tile as tile

    _patch_tile_drain()

    f32 = mybir.dt.float32
    u8 = mybir.dt.uint8
    mm_dt = getattr(mybir.dt, mm_dtype_name)
    Alu = mybir.AluOpType
    Act = mybir.ActivationFunctionType

    nc = bass.Bass()
    xt = nc.dram_tensor("xt", [IN_F, B_PER], u8, kind="ExternalInput")
    wt = nc.dram_tensor("wt", [IN_F, O_PER], f32, kind="ExternalInput")
    # 0/1 output is exact in uint8 - quarters the store DMA; host upcasts
    out = nc.dram_tensor("out", [B_PER, O_PER], u8, kind="ExternalOutput")

    with tile.TileContext(nc) as tc:
        with (
            tc.tile_pool(name="xu", bufs=1) as xu_pool,
            tc.tile_pool(name="xb", bufs=1) as xb_pool,
            tc.tile_pool(name="wr", bufs=4) as wr_pool,
            tc.tile_pool(name="ek", bufs=1) as ek_pool,
            tc.tile_pool(name="ps", bufs=1, space="PSUM") as ps_pool,
            tc.tile_pool(name="ot", bufs=12) as ot_pool,
        ):
            # touch Exp immediately so the ACT table load overlaps the
            # first input DMAs instead of the first exp
            warm = wr_pool.tile([P, 1], f32, name="warm", tag="warm")
            nc.vector.memset(warm[:], 0.0)
            nc.scalar.activation(warm[:], warm[:], Act.Exp)
            # PE p-state tickle: the sim drops the tensor clock back to the
            # slow ramp if the PE sits idle ~3us before its first matmul,
            # which costs ~1.9us across the first k-tiles.  A chain of DVE
            # ops delays a 1-element matmul to ~2us so the PE never looks
            # idle that long and every real matmul runs at full clock.
            td = xb_pool.tile([P, NH], f32, name="td", tag="td")
            nc.vector.memset(td[:], 0.0)
            nc.vector.tensor_scalar(td[:], td[:], 1.0, 0.0, Alu.mult, Alu.add)
            nc.vector.tensor_scalar(td[:], td[:], 1.0, 0.0, Alu.mult, Alu.add)

            ek = [
                ek_pool.tile([P, O_PER], mm_dt, name=f"e{k}", tag=f"e{k}")
                for k in range(KT)
            ]
            xb = [
                xb_pool.tile([P, B_PER], mm_dt, name=f"xb{k}", tag=f"xb{k}")
                for k in range(KT)
            ]

            # ---- input streams ----------------------------------------
            # SP order: [x k0, W-n0 k0 ramp, x k1, W-n0 k1, then x/W-n0
            # 2k-pairs] followed by the 16 per-k W-n1 DMAs.  Pairing keeps
            # SP's ~0.5-0.8us/DMA descriptor-gen off the critical path so
            # the n1 stream is fully issued before phase A's PE work ends.
            xup: list = []  # per-k APs into the x tiles

            def x_single(k):
                t = xu_pool.tile([P, B_PER], u8, name=f"xu{k}", tag=f"xu{k}")
                nc.sync.dma_start(t[:], xt[k * P : (k + 1) * P, :])
                xup.append(t[:])

            def x_dma(k_pair):
                if k_pair == 0:
                    x_single(0)
                    x_single(1)
                else:
                    t = xu_pool.tile(
                        [P, 2, B_PER], u8, name=f"xu{k_pair}", tag=f"xup{k_pair}"
                    )
                    r = k_pair * 2 * P
                    nc.sync.dma_start(t[:], xt[r : r + 2 * P, :])
                    xup.append(t[:, 0, :])
                    xup.append(t[:, 1, :])

            def w_n0_dma(k_pair):
                wrk = wr_pool.tile([P, 2, NH], f32, name="wrp", tag="wrp")
                r = k_pair * 2 * P
                nc.sync.dma_start(wrk[:], wt[r : r + 2 * P, 0:NH])
                for j in (0, 1):
                    nc.scalar.activation(
                        ek[k_pair * 2 + j][:, 0:NH], wrk[:, j, :], Act.Exp
                    )

            # SP order: x0 first (its DVE 2x-1 chain is longer), then k0's
            # W chunks, then x1/W1, then the 2k pairs - keeps both critical
            # paths (x -> lhsT, W -> exp -> rhs) tight at kernel start
            x_single(0)
            wr0 = wr_pool.tile([P, NH], f32, name="wr", tag="wr")
            for sl in (slice(0, 256), slice(256, NH)):
                nc.sync.dma_start(wr0[:, sl], wt[0:P, sl])
                nc.scalar.activation(ek[0][:, sl], wr0[:, sl], Act.Exp)
            x_single(1)
            wr1 = wr_pool.tile([P, NH], f32, name="wr", tag="wr")
            nc.sync.dma_start(wr1[:], wt[P : 2 * P, 0:NH])
            nc.scalar.activation(ek[1][:, 0:NH], wr1[:], Act.Exp)
            for kp in range(1, KT // 2):
                x_dma(kp)
                w_n0_dma(kp)

            # n1 half of W: streams behind the phase-A data, exp'd on ACT
            # as it lands; consumed only by phase B.  The 2k-pair DMAs park
            # rows 2p/2p+1 on partition p - a permutation of the contraction
            # index that phase A's x-pairs share; the n1 stream must use the
            # SAME pairing so phase B's lhsT/rhs partitions line up too.
            for k in (0, 1):
                wrk = wr_pool.tile([P, NH], f32, name="wr1", tag="wr1")
                nc.sync.dma_start(wrk[:], wt[k * P : (k + 1) * P, NH:])
                nc.scalar.activation(ek[k][:, NH:], wrk[:], Act.Exp)
            for kp in range(1, KT // 2):
                wrk = wr_pool.tile([P, 2, NH], f32, name="wr1p", tag="wr1p")
                r = kp * 2 * P
                nc.sync.dma_start(wrk[:], wt[r : r + 2 * P, NH:])
                for j in (0, 1):
                    nc.scalar.activation(ek[kp * 2 + j][:, NH:], wrk[:, j, :], Act.Exp)

            # x -> 2x-1 in {-1,+1} (exact in any fp dtype).  k0 split in
            # halves so m0..3's lhsT is ready a few hundred ns earlier.
            for k in range(KT):
                if k == 0:
                    nc.vector.tensor_scalar(
                        xb[k][:, 0:NH], xup[k][:, 0:NH], 2.0, 1.0,
                        Alu.mult, Alu.subtract,
                    )
                    nc.vector.tensor_scalar(
                        xb[k][:, NH:], xup[k][:, NH:], 2.0, 1.0,
                        Alu.mult, Alu.subtract,
                    )
                else:
                    nc.vector.tensor_scalar(
                        xb[k][:], xup[k][:], 2.0, 1.0, Alu.mult, Alu.subtract
                    )

            # ---- compute ----------------------------------------------
            def evict(m, col0, ncols, psrc):
                otm = ot_pool.tile([P, ncols], u8, name="ot", tag=f"ot{ncols}")
                nc.vector.tensor_scalar(otm[:], psrc, 0.0, None, Alu.is_gt)
                nc.sync.dma_start(
                    out[m * P : (m + 1) * P, col0 : col0 + ncols], otm[:]
                )

            # phase A: n-cols 0:512, k-outer over all 8 m (8 PSUM banks).
            # At k=15 each m's evict is queued right after its last matmul
            # so DVE drains the banks while PE finishes the k-tile.
            psA = {
                m: ps_pool.tile([P, NH], f32, name=f"ps{m}", tag=f"ps{m}")
                for m in range(MT)
            }
            # p-state tickle (see above): 1-element matmul after the DVE
            # delay chain; its psum garbage lands in psA[0][0,0] which the
            # k=0 start=True matmul immediately resets.
            nc.tensor.matmul(
                psA[0][0:1, 0:1], td[:, 0:1], td[:, 1:2],
                start=True, stop=True, skip_group_check=True,
            )
            for k in range(KT):
                if k == 0:
                    # k0 in 2x256-wide chunks so the first matmuls ride in
                    # right behind the two W-ramp exps.  start=True marks the
                    # WHOLE 2KB bank pending-zero, so only the first chunk
                    # starts; the second chunk's bytes are still pending-zero
                    # and its start=False write lands as an overwrite.
                    for c0 in (0, 256):
                        for m in range(MT):
                            nc.tensor.matmul(
                                psA[m][:, c0 : c0 + 256],
                                xb[0][:, m * P : (m + 1) * P],
                                ek[0][:, c0 : c0 + 256],
                                start=(c0 == 0),
                                stop=False,
                            )
                    continue
                for m in range(MT):
                    nc.tensor.matmul(
                        psA[m][:],
                        xb[k][:, m * P : (m + 1) * P],
                        ek[k][:, 0:NH],
                        start=False,
                        stop=(k == KT - 1),
                    )
                    if k == KT - 1:
                        evict(m, 0, NH, psA[m][:])

            # phase B: n-cols 512:1024, m-outer, all tiles resident.  The
            # last m runs as 2 narrow chunks in DIFFERENT banks (chunk 1
            # reuses m=0's long-evicted bank): a same-bank second chunk
            # would either re-zero chunk 0's results (start=True zeroes the
            # whole bank) or stall PE behind chunk 0's evict.  Both evicts
            # are emitted after all its matmuls so only the last one tails.
            for m in range(MT):
                nw = NH // tail_split if m == MT - 1 else NH
                chunks = []
                for ci, c0 in enumerate(range(0, NH, nw)):
                    psB = ps_pool.tile(
                        [P, nw], f32, name=f"ps{m}c{ci}", tag=f"ps{m if ci == 0 else 0}"
                    )
                    chunks.append((c0, psB))
                    for k in range(KT):
                        nc.tensor.matmul(
                            psB[:],
                            xb[k][:, m * P : (m + 1) * P],
                            ek[k][:, NH + c0 : NH + c0 + nw],
                            start=(k == 0),
                            stop=(k == KT - 1),
                        )
                for c0, psB in chunks:
                    evict(m, NH + c0, nw, psB[:])

    if split_waits:
        _split_multi_waits(nc)
    return nc


def _get_compiled(mm_dtype_name: str = "float32r", correction=False):
    key = (mm_dtype_name, correction)
    if key not in _COMPILED:
        _COMPILED[key] = _build(mm_dtype_name, correction=correction)
    return _COMPILED[key]


def kernel(x: np.ndarray, raw_weight: np.ndarray, _mm_dtype: str = "float32r",
           _correction=False, _trace: bool = False):
    from concourse.bass_utils import run_bass_kernel_spmd

    nc = _get_compiled(_mm_dtype, _correction)

    # materialize as numpy first (inputs may arrive as jax arrays)
    x = np.asarray(x)
    raw_weight = np.asarray(raw_weight)

    # x is exactly 0.0/1.0; uint8 encodes it losslessly and quarters the DMA
    xT = np.ascontiguousarray(x.T.astype(np.uint8))
    wT = np.ascontiguousarray(raw_weight.T).astype(np.float32, copy=False)

    in_maps = []
    for c in range(N_CORES):
        bg, og = divmod(c, OG)
        in_maps.append(
            {
                "xt": np.ascontiguousarray(xT[:, bg * B_PER : (bg + 1) * B_PER]),
                "wt": np.ascontiguousarray(wT[:, og * O_PER : (og + 1) * O_PER]),
            }
        )

    res = run_bass_kernel_spmd(
        nc, in_maps, core_ids=list(range(N_CORES)), trace=_trace
    )

    full = np.empty((BATCH, OUT_F), dtype=x.dtype)
    for c in range(N_CORES):
        bg, og = divmod(c, OG)
        full[bg * B_PER : (bg + 1) * B_PER, og * O_PER : (og + 1) * O_PER] = (
            res.results[c]["out"]
        )
    if _trace:
        kernel.last_results = res
    return full


# revision 16
# speedup vs baseline: 1.1495x; 1.0242x over previous
"""DigitalMapper kernel for 8 trn2 NeuronCores.

Math: reference computes  out = (x @ softmax(W, axis=1).T) > 0.5  with
x in {0,1}.  Let E = exp(W) (row-unnormalized).  Then

  out[b,o] > 0.5
    <=>  sum_i x[b,i]*E[o,i] / sum_i E[o,i] > 0.5
    <=>  sum_i (x[b,i] - 0.5) * E[o,i] > 0

so the softmax divide, the row-max subtraction and the per-column
threshold all fold into a single zero-threshold on a centered matmul.
(The row-max factor exp(m_o) scales a whole column positively - sign
is unchanged; |W| <= ~5.5 so exp never overflows fp32.)

Sharding: 4 batch-groups x 2 out-feature-groups across 8 cores.  Each
core gets x.T[:, bg*1024:...] and W.T[:, og*1024:...] (host does only
transpose/slice; subtract/exp/matmul/threshold all run on device) and
produces a [1024, 1024] block of the output.

Numerics: a single fp32r pass measures 256/8.4M sign flips vs the fp32
reference (rel err 7.8e-3, comfortably under the 2e-2 gate), so no
correction pass is run.  PE work is exactly 131072 rows = 54.6us/core.

Schedule (single-pass):
  phase A: out-cols 0:512 for ALL 8 m-tiles, k-outer, using all 8 PSUM
    banks.  Per k-tile it only needs x (128KB) + half of W (256KB), so
    DMA feeds it at ~1.1us/k vs 1.7us/k of PE work -> PE-bound (the old
    4m x 2n half-split needed the full W per k and was DMA-bound).
  The n1 half of W streams in behind phase A's data and is exp'd as it
    lands; x and W-n0 use 2-k-pair DMAs to halve SP descriptor-gen
    serialization so the n1 stream starts early enough.
  phase A's evicts interleave into the k=15 matmuls (no A->B bubble).
  phase B: out-cols 512:1024, m-outer over resident tiles, pipelined
    per-m evicts; the last m runs as 2x256-wide chunks so the final
    evict+store tail after the last matmul is short.
"""

import sys

sys.path.insert(0, "/opt/trn_rl_repo")

import numpy as np

BATCH, IN_F, OUT_F = 4096, 2048, 2048
N_CORES = 8
BG, OG = 4, 2  # batch groups x out-feature groups
B_PER = BATCH // BG  # 1024 batch rows per core
O_PER = OUT_F // OG  # 1024 out features per core
P = 128
KT = IN_F // P  # 16 contraction tiles
MT = B_PER // P  # 8 output row tiles per core
NH = 512  # out-col half width (one PSUM bank of fp32)

_COMPILED = {}


def _patch_tile_drain():
    """walrus in this container allows only ONE sem-wait per CTRL (Drain/NOP)
    instruction; Tile's kernel-tail drain aggregates one wait per live
    semaphore.  Split the waits across a chain of SP nops."""
    import concourse.mybir as mybir
    import concourse.tile as tile_mod
    from concourse.vector_clock import ScopedClock

    if getattr(tile_mod.TileContext, "_drain_split_patched", False):
        return

    def _drain_and_barrier_split(self, tick_clock, wait_clock):
        nc = self.nc
        drain_inst = nc.sync.drain()
        wait_clock.add_sem_waits(
            drain_inst.ins, ScopedClock({None: tick_clock.global_clock})
        )
        si = drain_inst.ins.sync_info
        waits = list(si.on_wait) if si is not None else []
        if len(waits) > 1:
            si.on_wait.clear()
            si.on_wait.extend(waits[:1])
            for w in waits[1:]:
                nop = nc.sync.nop(nofuse=True)
                if nop.ins.sync_info is None:
                    nop.ins.sync_info = mybir.SyncInfo(on_wait=[], on_update=[])
                nop.ins.sync_info.on_wait.append(w)
        nc.all_engine_barrier()
        assert self.sems is not None
        popped = nc._tile_sem_poison_stack.pop()
        assert popped is self._sem_poison
        nc.clear_and_free_semaphores(list(self.sems.allocated().values()))
        nc.all_engine_barrier()

    tile_mod.TileContext._drain_and_barrier = _drain_and_barrier_split
    tile_mod.TileContext._drain_split_patched = True


def _split_multi_waits(nc):
    """walrus here allows very few sem-waits per instruction.  Hoist extra
    waits onto same-engine NOPs placed immediately before the instruction
    (same blocking point, engine executes in order).  DMA-queue instructions
    keep their waits - their sync runs through the DGE queues."""
    import concourse.mybir as mybir

    n = 0
    for f in nc.m.functions:
        for bb in f.blocks:
            new_insts = []
            for inst in bb.instructions:
                si = inst.sync_info
                if si is not None and si.on_wait and len(si.on_wait) > 1:
                    waits = list(si.on_wait)
                    si.on_wait.clear()
                    si.on_wait.append(waits[0])
                    for w in waits[1:]:
                        n += 1
                        new_insts.append(
                            mybir.InstNoOp(
                                name=f"wsplit-{n}",
                                opcode="NoOp",
                                engine=inst.engine,
                                sync_info=mybir.SyncInfo(on_wait=[w], on_update=[]),
                                bass_nofuse=True,
                            )
                        )
                new_insts.append(inst)
            if n:
                try:
                    bb.instructions[:] = new_insts
                except TypeError:
                    bb.instructions = new_insts
    return n


def _build(mm_dtype_name: str = "float32r", split_waits: bool = True,
           repeats: int = 1, correction=False, grouped: bool = False,
           tail_split: int = 2):
    """One core's SPMD program: single mm_dtype pass, no correction.

    (The correction machinery from the 80us baseline was dropped: the
    rel-err gate is 2e-2 ~ 1678 sign flips, a lone fp32r pass measures
    256 flips on hw.  correction/grouped/repeats args are accepted for
    test.py compat and must be falsy/1.)
    """
    assert not correction and repeats == 1 and not grouped
    import concourse.bass as bass
    import concourse.mybir as mybir
    import concourse.tile as tile

    _patch_tile_drain()

    f32 = mybir.dt.float32
    u8 = mybir.dt.uint8
    mm_dt = getattr(mybir.dt, mm_dtype_name)
    Alu = mybir.AluOpType
    Act = mybir.ActivationFunctionType

    nc = bass.Bass()
    xt = nc.dram_tensor("xt", [IN_F, B_PER], u8, kind="ExternalInput")
    wt = nc.dram_tensor("wt", [IN_F, O_PER], f32, kind="ExternalInput")
    # 0/1 output is exact in uint8 - quarters the store DMA; host upcasts
    out = nc.dram_tensor("out", [B_PER, O_PER], u8, kind="ExternalOutput")

    with tile.TileContext(nc) as tc:
        with (
            tc.tile_pool(name="xu", bufs=1) as xu_pool,
            tc.tile_pool(name="xb", bufs=1) as xb_pool,
            tc.tile_pool(name="wr", bufs=4) as wr_pool,
            tc.tile_pool(name="ek", bufs=1) as ek_pool,
            tc.tile_pool(name="ps", bufs=1, space="PSUM") as ps_pool,
            tc.tile_pool(name="ot", bufs=12) as ot_pool,
        ):
            # touch Exp immediately so the ACT table load overlaps the
            # first input DMAs instead of the first exp
            warm = wr_pool.tile([P, 1], f32, name="warm", tag="warm")
            nc.vector.memset(warm[:], 0.0)
            nc.scalar.activation(warm[:], warm[:], Act.Exp)
            # PE p-state tickle: the sim drops the tensor clock back to the
            # slow ramp if the PE sits idle ~3us before its first matmul,
            # which costs ~1.9us across the first k-tiles.  A chain of DVE
            # ops delays a 1-element matmul to ~2us so the PE never looks
            # idle that long and every real matmul runs at full clock.
            td = xb_pool.tile([P, NH], f32, name="td", tag="td")
            nc.vector.memset(td[:], 0.0)
            nc.vector.tensor_scalar(td[:], td[:], 1.0, 0.0, Alu.mult, Alu.add)
            nc.vector.tensor_scalar(td[:], td[:], 1.0, 0.0, Alu.mult, Alu.add)

            ek = [
                ek_pool.tile([P, O_PER], mm_dt, name=f"e{k}", tag=f"e{k}")
                for k in range(KT)
            ]
            xb = [
                xb_pool.tile([P, B_PER], mm_dt, name=f"xb{k}", tag=f"xb{k}")
                for k in range(KT)
            ]

            # ---- input streams ----------------------------------------
            # SP order: [x k0, W-n0 k0 ramp, x k1, W-n0 k1, then x/W-n0
            # 2k-pairs] followed by the 16 per-k W-n1 DMAs.  Pairing keeps
            # SP's ~0.5-0.8us/DMA descriptor-gen off the critical path so
            # the n1 stream is fully issued before phase A's PE work ends.
            xup: list = []  # per-k APs into the x tiles

            def x_single(k):
                t = xu_pool.tile([P, B_PER], u8, name=f"xu{k}", tag=f"xu{k}")
                nc.sync.dma_start(t[:], xt[k * P : (k + 1) * P, :])
                xup.append(t[:])

            def x_dma(k_pair):
                if k_pair == 0:
                    x_single(0)
                    x_single(1)
                else:
                    t = xu_pool.tile(
                        [P, 2, B_PER], u8, name=f"xu{k_pair}", tag=f"xup{k_pair}"
                    )
                    r = k_pair * 2 * P
                    nc.sync.dma_start(t[:], xt[r : r + 2 * P, :])
                    xup.append(t[:, 0, :])
                    xup.append(t[:, 1, :])

            def w_n0_dma(k_pair):
                # Pool (SWDGE) issue: runs descriptor-gen in parallel with
                # SP's x/k0 stream, pulling k>=2's exp inputs ~1us earlier
                wrk = wr_pool.tile([P, 2, NH], f32, name="wrp", tag="wrp")
                r = k_pair * 2 * P
                nc.gpsimd.dma_start(wrk[:], wt[r : r + 2 * P, 0:NH])
                for j in (0, 1):
                    nc.scalar.activation(
                        ek[k_pair * 2 + j][:, 0:NH], wrk[:, j, :], Act.Exp
                    )

            # SP order: x0 first (its DVE 2x-1 chain is longer), then k0's
            # W chunks, then x1/W1, then the 2k pairs - keeps both critical
            # paths (x -> lhsT, W -> exp -> rhs) tight at kernel start
            x_single(0)
            wr0 = wr_pool.tile([P, NH], f32, name="wr", tag="wr")
            for sl in (slice(0, 256), slice(256, NH)):
                nc.sync.dma_start(wr0[:, sl], wt[0:P, sl])
                nc.scalar.activation(ek[0][:, sl], wr0[:, sl], Act.Exp)
            x_single(1)
            wr1 = wr_pool.tile([P, NH], f32, name="wr", tag="wr")
            nc.gpsimd.dma_start(wr1[:], wt[P : 2 * P, 0:NH])
            nc.scalar.activation(ek[1][:, 0:NH], wr1[:], Act.Exp)
            for kp in range(1, KT // 2):
                x_dma(kp)
                w_n0_dma(kp)

            # n1 half of W: streams behind the phase-A data, exp'd on ACT
            # as it lands; consumed only by phase B.  The 2k-pair DMAs park
            # rows 2p/2p+1 on partition p - a permutation of the contraction
            # index that phase A's x-pairs share; the n1 stream must use the
            # SAME pairing so phase B's lhsT/rhs partitions line up too.
            for k in (0, 1):
                wrk = wr_pool.tile([P, NH], f32, name="wr1", tag="wr1")
                nc.sync.dma_start(wrk[:], wt[k * P : (k + 1) * P, NH:])
                nc.scalar.activation(ek[k][:, NH:], wrk[:], Act.Exp)
            for kp in range(1, KT // 2):
                wrk = wr_pool.tile([P, 2, NH], f32, name="wr1p", tag="wr1p")
                r = kp * 2 * P
                nc.sync.dma_start(wrk[:], wt[r : r + 2 * P, NH:])
                for j in (0, 1):
                    nc.scalar.activation(ek[kp * 2 + j][:, NH:], wrk[:, j, :], Act.Exp)

            # x -> 2x-1 in {-1,+1} (exact in any fp dtype).  k0 split in
            # halves so m0..3's lhsT is ready a few hundred ns earlier.
            for k in range(KT):
                if k == 0:
                    nc.vector.tensor_scalar(
                        xb[k][:, 0:NH], xup[k][:, 0:NH], 2.0, 1.0,
                        Alu.mult, Alu.subtract,
                    )
                    nc.vector.tensor_scalar(
                        xb[k][:, NH:], xup[k][:, NH:], 2.0, 1.0,
                        Alu.mult, Alu.subtract,
                    )
                else:
                    nc.vector.tensor_scalar(
                        xb[k][:], xup[k][:], 2.0, 1.0, Alu.mult, Alu.subtract
                    )

            # ---- compute ----------------------------------------------
            def evict(m, col0, ncols, psrc):
                otm = ot_pool.tile([P, ncols], u8, name="ot", tag=f"ot{ncols}")
                nc.vector.tensor_scalar(otm[:], psrc, 0.0, None, Alu.is_gt)
                nc.sync.dma_start(
                    out[m * P : (m + 1) * P, col0 : col0 + ncols], otm[:]
                )

            # phase A: n-cols 0:512, k-outer over all 8 m (8 PSUM banks).
            # At k=15 each m's evict is queued right after its last matmul
            # so DVE drains the banks while PE finishes the k-tile.
            psA = {
                m: ps_pool.tile([P, NH], f32, name=f"ps{m}", tag=f"ps{m}")
                for m in range(MT)
            }
            # p-state tickle (see above): 1-element matmul after the DVE
            # delay chain; its psum garbage lands in psA[0][0,0] which the
            # k=0 start=True matmul immediately resets.
            nc.tensor.matmul(
                psA[0][0:1, 0:1], td[:, 0:1], td[:, 1:2],
                start=True, stop=True, skip_group_check=True,
            )
            for k in range(KT):
                if k == 0:
                    # k0 in 2x256-wide chunks so the first matmuls ride in
                    # right behind the two W-ramp exps.  start=True marks the
                    # WHOLE 2KB bank pending-zero, so only the first chunk
                    # starts; the second chunk's bytes are still pending-zero
                    # and its start=False write lands as an overwrite.
                    for c0 in (0, 256):
                        for m in range(MT):
                            nc.tensor.matmul(
                                psA[m][:, c0 : c0 + 256],
                                xb[0][:, m * P : (m + 1) * P],
                                ek[0][:, c0 : c0 + 256],
                                start=(c0 == 0),
                                stop=False,
                            )
                    continue
                for m in range(MT):
                    nc.tensor.matmul(
                        psA[m][:],
                        xb[k][:, m * P : (m + 1) * P],
                        ek[k][:, 0:NH],
                        start=False,
                        stop=(k == KT - 1),
                    )
                    if k == KT - 1:
                        evict(m, 0, NH, psA[m][:])

            # phase B: n-cols 512:1024, m-outer, all tiles resident.  The
            # last m runs as 2 narrow chunks in DIFFERENT banks (chunk 1
            # reuses m=0's long-evicted bank): a same-bank second chunk
            # would either re-zero chunk 0's results (start=True zeroes the
            # whole bank) or stall PE behind chunk 0's evict.  Both evicts
            # are emitted after all its matmuls so only the last one tails.
            for m in range(MT):
                nw = NH // tail_split if m == MT - 1 else NH
                chunks = []
                for ci, c0 in enumerate(range(0, NH, nw)):
                    psB = ps_pool.tile(
                        [P, nw], f32, name=f"ps{m}c{ci}", tag=f"ps{m if ci == 0 else 0}"
                    )
                    chunks.append((c0, psB))
                    for k in range(KT):
                        nc.tensor.matmul(
                            psB[:],
                            xb[k][:, m * P : (m + 1) * P],
                            ek[k][:, NH + c0 : NH + c0 + nw],
                            start=(k == 0),
                            stop=(k == KT - 1),
                        )
                for c0, psB in chunks:
                    evict(m, NH + c0, nw, psB[:])

    if split_waits:
        _split_multi_waits(nc)
    return nc


def _get_compiled(mm_dtype_name: str = "float32r", correction=False):
    key = (mm_dtype_name, correction)
    if key not in _COMPILED:
        _COMPILED[key] = _build(mm_dtype_name, correction=correction)
    return _COMPILED[key]


def kernel(x: np.ndarray, raw_weight: np.ndarray, _mm_dtype: str = "float32r",
           _correction=False, _trace: bool = False):
    from concourse.bass_utils import run_bass_kernel_spmd

    nc = _get_compiled(_mm_dtype, _correction)

    # materialize as numpy first (inputs may arrive as jax arrays)
    x = np.asarray(x)
    raw_weight = np.asarray(raw_weight)

    # x is exactly 0.0/1.0; uint8 encodes it losslessly and quarters the DMA
    xT = np.ascontiguousarray(x.T.astype(np.uint8))
    wT = np.ascontiguousarray(raw_weight.T).astype(np.float32, copy=False)

    in_maps = []
    for c in range(N_CORES):
        bg, og = divmod(c, OG)
        in_maps.append(
            {
                "xt": np.ascontiguousarray(xT[:, bg * B_PER : (bg + 1) * B_PER]),
                "wt": np.ascontiguousarray(wT[:, og * O_PER : (og + 1) * O_PER]),
            }
        )

    res = run_bass_kernel_spmd(
        nc, in_maps, core_ids=list(range(N_CORES)), trace=_trace
    )

    full = np.empty((BATCH, OUT_F), dtype=x.dtype)
    for c in range(N_CORES):
        bg, og = divmod(c, OG)
        full[bg * B_PER : (bg + 1) * B_PER, og * O_PER : (og + 1) * O_PER] = (
            res.results[c]["out"]
        )
    if _trace:
        kernel.last_results = res
    return full


# revision 22
# speedup vs baseline: 1.1589x; 1.0082x over previous
"""DigitalMapper kernel for 8 trn2 NeuronCores.

Math: reference computes  out = (x @ softmax(W, axis=1).T) > 0.5  with
x in {0,1}.  Let E = exp(W) (row-unnormalized).  Then

  out[b,o] > 0.5
    <=>  sum_i x[b,i]*E[o,i] / sum_i E[o,i] > 0.5
    <=>  sum_i (x[b,i] - 0.5) * E[o,i] > 0

so the softmax divide, the row-max subtraction and the per-column
threshold all fold into a single zero-threshold on a centered matmul.
(The row-max factor exp(m_o) scales a whole column positively - sign
is unchanged; |W| <= ~5.5 so exp never overflows fp32.)

Sharding: 4 batch-groups x 2 out-feature-groups across 8 cores.  Each
core gets x.T[:, bg*1024:...] and W.T[:, og*1024:...] (host does only
transpose/slice; subtract/exp/matmul/threshold all run on device) and
produces a [1024, 1024] block of the output.

Numerics: a single fp32r pass measures 256/8.4M sign flips vs the fp32
reference (rel err 7.8e-3, comfortably under the 2e-2 gate), so no
correction pass is run.  PE work is exactly 131072 rows = 54.6us/core.

Schedule (single-pass):
  phase A: out-cols 0:512 for ALL 8 m-tiles, k-outer, using all 8 PSUM
    banks.  Per k-tile it only needs x (128KB) + half of W (256KB), so
    DMA feeds it at ~1.1us/k vs 1.7us/k of PE work -> PE-bound (the old
    4m x 2n half-split needed the full W per k and was DMA-bound).
  The n1 half of W streams in behind phase A's data and is exp'd as it
    lands; x and W-n0 use 2-k-pair DMAs to halve SP descriptor-gen
    serialization so the n1 stream starts early enough.
  phase A's evicts interleave into the k=15 matmuls (no A->B bubble).
  phase B: out-cols 512:1024, m-outer over resident tiles, pipelined
    per-m evicts; the last m runs as 2x256-wide chunks so the final
    evict+store tail after the last matmul is short.
"""

import sys

sys.path.insert(0, "/opt/trn_rl_repo")

import numpy as np

BATCH, IN_F, OUT_F = 4096, 2048, 2048
N_CORES = 8
BG, OG = 4, 2  # batch groups x out-feature groups
B_PER = BATCH // BG  # 1024 batch rows per core
O_PER = OUT_F // OG  # 1024 out features per core
P = 128
KT = IN_F // P  # 16 contraction tiles
MT = B_PER // P  # 8 output row tiles per core
NH = 512  # out-col half width (one PSUM bank of fp32)

_COMPILED = {}


def _patch_tile_drain():
    """walrus in this container allows only ONE sem-wait per CTRL (Drain/NOP)
    instruction; Tile's kernel-tail drain aggregates one wait per live
    semaphore.  Split the waits across a chain of SP nops."""
    import concourse.mybir as mybir
    import concourse.tile as tile_mod
    from concourse.vector_clock import ScopedClock

    if getattr(tile_mod.TileContext, "_drain_split_patched", False):
        return

    def _drain_and_barrier_split(self, tick_clock, wait_clock):
        nc = self.nc
        drain_inst = nc.sync.drain()
        wait_clock.add_sem_waits(
            drain_inst.ins, ScopedClock({None: tick_clock.global_clock})
        )
        si = drain_inst.ins.sync_info
        waits = list(si.on_wait) if si is not None else []
        if len(waits) > 1:
            si.on_wait.clear()
            si.on_wait.extend(waits[:1])
            for w in waits[1:]:
                nop = nc.sync.nop(nofuse=True)
                if nop.ins.sync_info is None:
                    nop.ins.sync_info = mybir.SyncInfo(on_wait=[], on_update=[])
                nop.ins.sync_info.on_wait.append(w)
        nc.all_engine_barrier()
        assert self.sems is not None
        popped = nc._tile_sem_poison_stack.pop()
        assert popped is self._sem_poison
        nc.clear_and_free_semaphores(list(self.sems.allocated().values()))
        nc.all_engine_barrier()

    tile_mod.TileContext._drain_and_barrier = _drain_and_barrier_split
    tile_mod.TileContext._drain_split_patched = True


def _split_multi_waits(nc):
    """walrus here allows very few sem-waits per instruction.  Hoist extra
    waits onto same-engine NOPs placed immediately before the instruction
    (same blocking point, engine executes in order).  DMA-queue instructions
    keep their waits - their sync runs through the DGE queues."""
    import concourse.mybir as mybir

    n = 0
    for f in nc.m.functions:
        for bb in f.blocks:
            new_insts = []
            for inst in bb.instructions:
                si = inst.sync_info
                if si is not None and si.on_wait and len(si.on_wait) > 1:
                    waits = list(si.on_wait)
                    si.on_wait.clear()
                    si.on_wait.append(waits[0])
                    for w in waits[1:]:
                        n += 1
                        new_insts.append(
                            mybir.InstNoOp(
                                name=f"wsplit-{n}",
                                opcode="NoOp",
                                engine=inst.engine,
                                sync_info=mybir.SyncInfo(on_wait=[w], on_update=[]),
                                bass_nofuse=True,
                            )
                        )
                new_insts.append(inst)
            if n:
                try:
                    bb.instructions[:] = new_insts
                except TypeError:
                    bb.instructions = new_insts
    return n


def _build(mm_dtype_name: str = "float32r", split_waits: bool = True,
           repeats: int = 1, correction=False, grouped: bool = False,
           tail_split: int = 2):
    """One core's SPMD program: single mm_dtype pass, no correction.

    (The correction machinery from the 80us baseline was dropped: the
    rel-err gate is 2e-2 ~ 1678 sign flips, a lone fp32r pass measures
    256 flips on hw.  correction/grouped/repeats args are accepted for
    test.py compat and must be falsy/1.)
    """
    assert not correction and repeats == 1 and not grouped
    import concourse.bass as bass
    import concourse.mybir as mybir
    import concourse.tile as tile

    _patch_tile_drain()

    f32 = mybir.dt.float32
    u8 = mybir.dt.uint8
    mm_dt = getattr(mybir.dt, mm_dtype_name)
    Alu = mybir.AluOpType
    Act = mybir.ActivationFunctionType

    nc = bass.Bass()
    xt = nc.dram_tensor("xt", [IN_F, B_PER], u8, kind="ExternalInput")
    wt = nc.dram_tensor("wt", [IN_F, O_PER], f32, kind="ExternalInput")
    # 0/1 output is exact in uint8 - quarters the store DMA; host upcasts
    out = nc.dram_tensor("out", [B_PER, O_PER], u8, kind="ExternalOutput")

    with tile.TileContext(nc) as tc:
        with (
            tc.tile_pool(name="xu", bufs=1) as xu_pool,
            tc.tile_pool(name="xb", bufs=1) as xb_pool,
            tc.tile_pool(name="wr", bufs=3) as wr_pool,
            tc.tile_pool(name="ek", bufs=1) as ek_pool,
            tc.tile_pool(name="ps", bufs=1, space="PSUM") as ps_pool,
            tc.tile_pool(name="ot", bufs=6) as ot_pool,
        ):
            # touch Exp immediately so the ACT table load overlaps the
            # first input DMAs instead of the first exp
            warm = wr_pool.tile([P, 1], f32, name="warm", tag="warm")
            nc.vector.memset(warm[:], 0.0)
            nc.scalar.activation(warm[:], warm[:], Act.Exp)
            # PE p-state tickle: the sim drops the tensor clock back to the
            # slow ramp if the PE sits idle ~3us before its first matmul,
            # which costs ~1.9us across the first k-tiles.  A chain of DVE
            # ops delays a 1-element matmul to ~2us so the PE never looks
            # idle that long and every real matmul runs at full clock.
            td = xb_pool.tile([P, NH], f32, name="td", tag="td")
            nc.vector.memset(td[:], 0.0)
            nc.vector.tensor_scalar(td[:], td[:], 1.0, 0.0, Alu.mult, Alu.add)
            nc.vector.tensor_scalar(td[:], td[:], 1.0, 0.0, Alu.mult, Alu.add)

            ek = [
                ek_pool.tile([P, O_PER], mm_dt, name=f"e{k}", tag=f"e{k}")
                for k in range(KT)
            ]
            xb = [
                xb_pool.tile([P, B_PER], mm_dt, name=f"xb{k}", tag=f"xb{k}")
                for k in range(KT)
            ]

            # ---- input streams ----------------------------------------
            # SP order: [x k0, W-n0 k0 ramp, x k1, W-n0 k1, then x/W-n0
            # 2k-pairs] followed by the 16 per-k W-n1 DMAs.  Pairing keeps
            # SP's ~0.5-0.8us/DMA descriptor-gen off the critical path so
            # the n1 stream is fully issued before phase A's PE work ends.
            xup: list = []  # per-k APs into the x tiles

            def x_single(k):
                t = xu_pool.tile([P, B_PER], u8, name=f"xu{k}", tag=f"xu{k}")
                nc.sync.dma_start(t[:], xt[k * P : (k + 1) * P, :])
                xup.append(t[:])

            def x_dma(k_pair):
                if k_pair == 0:
                    x_single(0)
                    x_single(1)
                else:
                    t = xu_pool.tile(
                        [P, 2, B_PER], u8, name=f"xu{k_pair}", tag=f"xup{k_pair}"
                    )
                    r = k_pair * 2 * P
                    nc.sync.dma_start(t[:], xt[r : r + 2 * P, :])
                    xup.append(t[:, 0, :])
                    xup.append(t[:, 1, :])

            def w_n0_dma(k_pair):
                # Pool (SWDGE) issue: runs descriptor-gen in parallel with
                # SP's x/k0 stream, pulling k>=2's exp inputs ~1us earlier
                wrk = wr_pool.tile([P, 2, NH], f32, name="wrp", tag="wrp")
                r = k_pair * 2 * P
                nc.gpsimd.dma_start(wrk[:], wt[r : r + 2 * P, 0:NH])
                for j in (0, 1):
                    nc.scalar.activation(
                        ek[k_pair * 2 + j][:, 0:NH], wrk[:, j, :], Act.Exp
                    )

            # SP order: x0 first (its DVE 2x-1 chain is longer), then k0's
            # W chunks, then x1/W1, then the 2k pairs - keeps both critical
            # paths (x -> lhsT, W -> exp -> rhs) tight at kernel start
            x_single(0)
            wr0 = wr_pool.tile([P, NH], f32, name="wr", tag="wr")
            for sl in (slice(0, 256), slice(256, NH)):
                nc.sync.dma_start(wr0[:, sl], wt[0:P, sl])
                nc.scalar.activation(ek[0][:, sl], wr0[:, sl], Act.Exp)
            x_single(1)
            wr1 = wr_pool.tile([P, NH], f32, name="wr", tag="wr")
            nc.gpsimd.dma_start(wr1[:], wt[P : 2 * P, 0:NH])
            nc.scalar.activation(ek[1][:, 0:NH], wr1[:], Act.Exp)
            for kp in range(1, KT // 2):
                x_dma(kp)
                w_n0_dma(kp)

            # n1 half of W: streams behind the phase-A data, exp'd on ACT
            # as it lands; consumed only by phase B.  The 2k-pair DMAs park
            # rows 2p/2p+1 on partition p - a permutation of the contraction
            # index that phase A's x-pairs share; the n1 stream must use the
            # SAME pairing so phase B's lhsT/rhs partitions line up too.
            # ek16: fp16 copies of the last 256 out-cols' exp(W), letting the
            # final two chunks of the last m-tile run 128-wide at 1 cyc/row
            # (fp32r needs >=256 free) - shortens the end-of-kernel evict
            # tail.  fp16 on 0.4% of outputs adds ~0 flips.
            f16 = mybir.dt.float16
            ek16 = [
                ek_pool.tile([P, 256], f16, name=f"e16_{k}", tag=f"e16_{k}")
                for k in range(KT)
            ]
            for k in (0, 1):
                wrk = wr_pool.tile([P, NH], f32, name="wr1", tag="wr1")
                nc.sync.dma_start(wrk[:], wt[k * P : (k + 1) * P, NH:])
                nc.scalar.activation(ek[k][:, NH:], wrk[:], Act.Exp)
                nc.scalar.activation(ek16[k][:], wrk[:, 256:NH], Act.Exp)
            for kp in range(1, KT // 2):
                wrk = wr_pool.tile([P, 2, NH], f32, name="wr1p", tag="wr1p")
                r = kp * 2 * P
                nc.sync.dma_start(wrk[:], wt[r : r + 2 * P, NH:])
                for j in (0, 1):
                    nc.scalar.activation(ek[kp * 2 + j][:, NH:], wrk[:, j, :], Act.Exp)
                    nc.scalar.activation(
                        ek16[kp * 2 + j][:], wrk[:, j, 256:NH], Act.Exp
                    )

            # x -> 2x-1 in {-1,+1} (exact in any fp dtype).  k0 split in
            # halves so m0..3's lhsT is ready a few hundred ns earlier.
            for k in range(KT):
                if k == 0:
                    nc.vector.tensor_scalar(
                        xb[k][:, 0:NH], xup[k][:, 0:NH], 2.0, 1.0,
                        Alu.mult, Alu.subtract,
                    )
                    nc.vector.tensor_scalar(
                        xb[k][:, NH:], xup[k][:, NH:], 2.0, 1.0,
                        Alu.mult, Alu.subtract,
                    )
                else:
                    nc.vector.tensor_scalar(
                        xb[k][:], xup[k][:], 2.0, 1.0, Alu.mult, Alu.subtract
                    )
            # fp16 lhsT for the last m-tile (pairs with ek16; +-1 is exact)
            xb16 = []
            mlast = (MT - 1) * P
            for k in range(KT):
                t = xb_pool.tile([P, P], f16, name=f"xb16_{k}", tag=f"xb16_{k}")
                nc.vector.tensor_scalar(
                    t[:], xup[k][:, mlast : mlast + P], 2.0, 1.0,
                    Alu.mult, Alu.subtract,
                )
                xb16.append(t)

            # ---- compute ----------------------------------------------
            def evict(m, col0, ncols, psrc):
                otm = ot_pool.tile([P, ncols], u8, name="ot", tag=f"ot{ncols}")
                nc.vector.tensor_scalar(otm[:], psrc, 0.0, None, Alu.is_gt)
                nc.sync.dma_start(
                    out[m * P : (m + 1) * P, col0 : col0 + ncols], otm[:]
                )

            # phase A: n-cols 0:512, k-outer over all 8 m (8 PSUM banks).
            # At k=15 each m's evict is queued right after its last matmul
            # so DVE drains the banks while PE finishes the k-tile.
            psA = {
                m: ps_pool.tile([P, NH], f32, name=f"ps{m}", tag=f"ps{m}")
                for m in range(MT)
            }
            # p-state tickle (see above): 1-element matmul after the DVE
            # delay chain; its psum garbage lands in psA[0][0,0] which the
            # k=0 start=True matmul immediately resets.
            nc.tensor.matmul(
                psA[0][0:1, 0:1], td[:, 0:1], td[:, 1:2],
                start=True, stop=True, skip_group_check=True,
            )
            for k in range(KT):
                if k == 0:
                    # k0 in 2x256-wide chunks so the first matmuls ride in
                    # right behind the two W-ramp exps.  start=True marks the
                    # WHOLE 2KB bank pending-zero, so only the first chunk
                    # starts; the second chunk's bytes are still pending-zero
                    # and its start=False write lands as an overwrite.
                    for c0 in (0, 256):
                        for m in range(MT):
                            nc.tensor.matmul(
                                psA[m][:, c0 : c0 + 256],
                                xb[0][:, m * P : (m + 1) * P],
                                ek[0][:, c0 : c0 + 256],
                                start=(c0 == 0),
                                stop=False,
                            )
                    continue
                for m in range(MT):
                    nc.tensor.matmul(
                        psA[m][:],
                        xb[k][:, m * P : (m + 1) * P],
                        ek[k][:, 0:NH],
                        start=False,
                        stop=(k == KT - 1),
                    )
                    if k == KT - 1:
                        evict(m, 0, NH, psA[m][:])

            # phase B: n-cols 512:1024, m-outer, all tiles resident.  The
            # last m runs as 2 narrow chunks in DIFFERENT banks (chunk 1
            # reuses m=0's long-evicted bank): a same-bank second chunk
            # would either re-zero chunk 0's results (start=True zeroes the
            # whole bank) or stall PE behind chunk 0's evict.  Both evicts
            # are emitted after all its matmuls so only the last one tails.
            for m in range(MT - 1):
                psB = ps_pool.tile([P, NH], f32, name=f"psB{m}", tag=f"ps{m}")
                for k in range(KT):
                    nc.tensor.matmul(
                        psB[:],
                        xb[k][:, m * P : (m + 1) * P],
                        ek[k][:, NH:],
                        start=(k == 0),
                        stop=(k == KT - 1),
                    )
                evict(m, NH, NH, psB[:])
            # last m-tile: 256-wide fp32r chunk + 2x128-wide fp16 chunks,
            # each in its own (long-evicted) bank so the k-loops run
            # back-to-back and only the final 128-wide evict chain tails.
            m = MT - 1
            chunks = []
            for ci, (c0, nw, bank) in enumerate(
                ((0, 256, m), (256, P, 0), (256 + P, P, 1))
            ):
                psB = ps_pool.tile([P, nw], f32, name=f"ps{m}c{ci}", tag=f"ps{bank}")
                chunks.append((c0, nw, psB))
                for k in range(KT):
                    if nw == 256:
                        lhsT, rhs = xb[k][:, m * P :], ek[k][:, NH : NH + 256]
                    else:
                        lhsT = xb16[k][:]
                        rhs = ek16[k][:, c0 - 256 : c0 - 256 + P]
                    nc.tensor.matmul(
                        psB[:], lhsT, rhs, start=(k == 0), stop=(k == KT - 1)
                    )
            for c0, nw, psB in chunks:
                evict(m, NH + c0, nw, psB[:])

    if split_waits:
        _split_multi_waits(nc)
    return nc


def _get_compiled(mm_dtype_name: str = "float32r", correction=False):
    key = (mm_dtype_name, correction)
    if key not in _COMPILED:
        _COMPILED[key] = _build(mm_dtype_name, correction=correction)
    return _COMPILED[key]


def kernel(x: np.ndarray, raw_weight: np.ndarray, _mm_dtype: str = "float32r",
           _correction=False, _trace: bool = False):
    from concourse.bass_utils import run_bass_kernel_spmd

    nc = _get_compiled(_mm_dtype, _correction)

    # materialize as numpy first (inputs may arrive as jax arrays)
    x = np.asarray(x)
    raw_weight = np.asarray(raw_weight)

    # x is exactly 0.0/1.0; uint8 encodes it losslessly and quarters the DMA
    xT = np.ascontiguousarray(x.T.astype(np.uint8))
    wT = np.ascontiguousarray(raw_weight.T).astype(np.float32, copy=False)

    in_maps = []
    for c in range(N_CORES):
        bg, og = divmod(c, OG)
        in_maps.append(
            {
                "xt": np.ascontiguousarray(xT[:, bg * B_PER : (bg + 1) * B_PER]),
                "wt": np.ascontiguousarray(wT[:, og * O_PER : (og + 1) * O_PER]),
            }
        )

    res = run_bass_kernel_spmd(
        nc, in_maps, core_ids=list(range(N_CORES)), trace=_trace
    )

    full = np.empty((BATCH, OUT_F), dtype=x.dtype)
    for c in range(N_CORES):
        bg, og = divmod(c, OG)
        full[bg * B_PER : (bg + 1) * B_PER, og * O_PER : (og + 1) * O_PER] = (
            res.results[c]["out"]
        )
    if _trace:
        kernel.last_results = res
    return full
